# revision 13
# baseline (speedup 1.0000x reference)
"""NT-Xent loss kernel for Trainium2 (8 NeuronCores, SPMD).

Math (derived from the reference):
  z = concat(z_i, z_j)                         (N=8192, D=128)
  zn = z / max(||z||, 1e-8)
  G[a,b] = zn[a].zn[b] / temp   (temp=0.5, so G = 2*cos-sim)
  per-row loss_a = log(sum_{i != p(a)} e^{G[a,i]}) - G[a,a]
                 = log(sum_all_i e^{G[a,i]} - e^{G[a,p(a)]}) - G[a,a]
  with p(a) = (a + 4096) % 8192;  loss = mean over rows.

Sharding: data-parallel row blocks.  Core c handles rows
[1024c, 1024c+1024).  Each core receives the FULL z, but np.roll'd by
-1024c rows so the SPMD program only uses static offsets: its row block
is always columns [0, 1024) of the (rotated) transposed operand, the
"diag" entry of block-row j is column j, and the masked positive column
is j+4096.  No collectives; the host sums 8 partial outputs.

Per-core device pipeline:
  1. DMA z_nat [8192,128] (tiled [128, 64, 128]) and zT [128, 8192].
  2. n2[p,t] = ||row 128t+p||^2 via DVE tensor_tensor_reduce / ACT
     Square+accum (split across both engines).
  3. r = n2^-0.5 as exp(-0.5*ln(n2)) (both funcs in one ACT table set).
  4. r transposed (PE) + flattened (DMA) into a row vector r_row[1,8192];
     K=1 matmul broadcasts it across partitions; DVE multiplies zT
     chunks -> znT (normalized, transposed operand).
  5. For each 128-row tile: PE matmul (fp32r) -> PSUM [128, 2048] blocks;
     ACT Exp(scale=2) with accum_out gives row sums of e^G; DVE
     mask-reduce extracts 2*diag and 2*pos from PSUM.
  6. loss_tile = ln(S - e^pos2) - diag2, DMA'd out as [128, 8] per core.
"""

import os

import numpy as np

N = 8192
D = 128
NCORES = 8
RPC = N // NCORES  # rows per core = 1024
NT = N // 128  # 64 z_nat tiles
RT = RPC // 128  # 8 row tiles per core
CHUNK = 512
NCHUNK = N // CHUNK  # 16
CB = 2048  # psum col-block for exp
NCB = N // CB  # 4
HALF = N // 2

_NC_CACHE = {}


def _build_nc(mm_dtype: str, reps: int = 1):
    from contextlib import ExitStack

    import concourse.bass as bass  # noqa: F401
    import concourse.tile as tile
    from concourse import bacc, mybir
    from concourse.masks import make_identity

    f32 = mybir.dt.float32
    bf16 = mybir.dt.bfloat16
    FN = mybir.ActivationFunctionType
    ALU = mybir.AluOpType

    if mm_dtype == "f32r":
        mmdt = mybir.dt.float32r
    elif mm_dtype == "f32":
        mmdt = f32
    elif mm_dtype == "bf16":
        mmdt = bf16
    else:
        raise ValueError(mm_dtype)

    nc = bacc.Bacc(
        "TRN2", target_bir_lowering=False, debug=False, num_devices=NCORES
    )
    z_nat = nc.dram_tensor("z_nat", [N, D], f32, kind="ExternalInput")
    zT = nc.dram_tensor("zT", [D, N], f32, kind="ExternalInput")
    loss_out = nc.dram_tensor("loss", [128, RT], f32, kind="ExternalOutput")

    def body(pools, ident, ones_row):
        big, small, scr, psum = pools

        # ---- input DMAs (4 x 1MB each tensor) ----
        znat_sb = []
        z_nat_t = z_nat.ap().rearrange("(t p) d -> p t d", p=128)
        for q in range(4):
            t = big.tile([128, 16, 128], f32, tag=f"znat{q}")
            nc.sync.dma_start(t[:], z_nat_t[:, q * 16 : (q + 1) * 16, :])
            znat_sb.append(t)
        zt_sb = []
        for q in range(4):
            t = big.tile([128, CB], f32, tag=f"zt{q}")
            nc.sync.dma_start(t[:], zT.ap()[:, q * CB : (q + 1) * CB])
            zt_sb.append(t)

        # ---- n2 per row, natural layout [128, 64] ----
        n2 = small.tile([128, NT], f32, tag="n2")
        ttr_scr = scr.tile([128, 128], f32, tag="ttr_scr")
        sq_scr = scr.tile([128, 128], f32, tag="sq_scr")
        for t in range(NT):
            q, i = divmod(t, 16)
            src = znat_sb[q][:, i, :]
            if t % 2 == 0:
                nc.vector.scalar_tensor_tensor(
                    out=ttr_scr[:],
                    in0=src,
                    scalar=1.0,
                    in1=src,
                    op0=ALU.mult,
                    op1=ALU.mult,
                    accum_out=n2[:, t : t + 1],
                )
            else:
                nc.scalar.activation(
                    out=sq_scr[:],
                    in_=src,
                    func=FN.Square,
                    accum_out=n2[:, t : t + 1],
                )

        # ---- r = exp(-0.5 * ln(max(n2, 1e-16))) = 1/sqrt(n2) ----
        nmax = small.tile([128, NT], f32, tag="nmax")
        nc.vector.tensor_scalar_max(out=nmax[:], in0=n2[:], scalar1=1e-16)
        lnn = small.tile([128, NT], f32, tag="lnn")
        nc.scalar.activation(out=lnn[:], in_=nmax[:], func=FN.Ln)
        r_nat = small.tile([128, NT], f32, tag="r_nat")
        nc.scalar.activation(out=r_nat[:], in_=lnn[:], func=FN.Exp, scale=-0.5)

        # ---- r_nat [128,64] -> r_row [1, 8192] (transpose + flatten) ----
        rT_ps = psum.tile([64, 128], f32, tag="ps")
        nc.tensor.transpose(rT_ps[:], r_nat[:], ident[:])
        rT = small.tile([64, 128], f32, tag="rT")
        nc.vector.tensor_copy(out=rT[:], in_=rT_ps[:])
        r_row = small.tile([1, N], f32, tag="r_row")
        nc.sync.dma_start(r_row[:], rT[:])

        # ---- normalize: znT chunks = zT * broadcast(r_row) ----
        znt = []
        for ct in range(NCHUNK):
            R_ps = psum.tile([128, CHUNK], f32, tag="ps")
            nc.tensor.matmul(
                R_ps[:],
                ones_row[:, :],
                r_row[:, ct * CHUNK : (ct + 1) * CHUNK],
                start=True,
                stop=True,
            )
            zn_c = big.tile([128, CHUNK], mmdt, tag=f"znt{ct}")
            q, off = divmod(ct * CHUNK, CB)
            nc.vector.tensor_mul(
                zn_c[:], zt_sb[q][:, off : off + CHUNK], R_ps[:]
            )
            znt.append(zn_c)

        # ---- main: G row blocks, exp row-sums, diag/pos extraction ----
        accs = small.tile([128, RT, NCB], f32, tag="accs")
        diag2 = small.tile([128, RT], f32, tag="diag2")
        pos2 = small.tile([128, RT], f32, tag="pos2")
        e_scr = scr.tile([128, CB], bf16, tag="e_scr")
        m_scr = scr.tile([128, 128], f32, tag="m_scr")

        for rt in range(RT):
            lct, loff = divmod(rt * 128, CHUNK)
            lhsT = znt[lct][:, loff : loff + 128]
            for cb in range(NCB):
                G_ps = psum.tile([128, CB], f32, tag="ps")
                for k in range(4):
                    ct = cb * 4 + k
                    nc.tensor.matmul(
                        G_ps[:, k * CHUNK : (k + 1) * CHUNK],
                        lhsT,
                        znt[ct][:],
                        start=True,
                        stop=True,
                    )
                if cb == 0:
                    nc.vector.scalar_tensor_tensor(
                        out=m_scr[:],
                        in0=G_ps[:, rt * 128 : rt * 128 + 128],
                        scalar=2.0,
                        in1=ident[:],
                        op0=ALU.mult,
                        op1=ALU.mult,
                        accum_out=diag2[:, rt : rt + 1],
                    )
                if cb == 2:
                    off = HALF + rt * 128 - 2 * CB
                    nc.vector.scalar_tensor_tensor(
                        out=m_scr[:],
                        in0=G_ps[:, off : off + 128],
                        scalar=2.0,
                        in1=ident[:],
                        op0=ALU.mult,
                        op1=ALU.mult,
                        accum_out=pos2[:, rt : rt + 1],
                    )
                nc.scalar.activation(
                    out=e_scr[:],
                    in_=G_ps[:],
                    func=FN.Exp,
                    scale=2.0,
                    accum_out=accs[:, rt, cb : cb + 1],
                )

        # ---- assembly: loss = ln(S - e^pos2) - diag2 ----
        S = small.tile([128, RT], f32, tag="S")
        nc.vector.reduce_sum(out=S[:], in_=accs[:], axis=mybir.AxisListType.X)
        P = small.tile([128, RT], f32, tag="P")
        nc.scalar.activation(out=P[:], in_=pos2[:], func=FN.Exp)
        T = small.tile([128, RT], f32, tag="T")
        nc.vector.tensor_sub(T[:], S[:], P[:])
        L = small.tile([128, RT], f32, tag="L")
        nc.scalar.activation(out=L[:], in_=T[:], func=FN.Ln)
        loss_sb = small.tile([128, RT], f32, tag="loss_sb")
        nc.vector.tensor_sub(loss_sb[:], L[:], diag2[:])
        nc.sync.dma_start(loss_out.ap()[:, :], loss_sb[:])

    with tile.TileContext(nc) as tc, ExitStack() as ctx:
        const = ctx.enter_context(tc.tile_pool(name="const", bufs=1))
        big = ctx.enter_context(tc.tile_pool(name="big", bufs=1))
        small = ctx.enter_context(tc.tile_pool(name="small", bufs=1))
        scr = ctx.enter_context(tc.tile_pool(name="scr", bufs=1))
        psum = ctx.enter_context(tc.tile_pool(name="psum", bufs=2, space="PSUM"))

        ident = const.tile([128, 128], f32, tag="ident")
        make_identity(nc, ident[:])
        ones_row = const.tile([1, 128], f32, tag="ones_row")
        nc.vector.memset(ones_row[:], 1.0)

        for _rep in range(reps):
            body((big, small, scr, psum), ident, ones_row)

    nc.compile()
    return nc


def get_nc(reps: int = 1):
    mm_dtype = os.environ.get("NTX_MM_DTYPE", "f32r")
    key = (mm_dtype, reps)
    if key not in _NC_CACHE:
        _NC_CACHE[key] = _build_nc(mm_dtype, reps)
    return _NC_CACHE[key]


def make_in_maps(z_i: np.ndarray, z_j: np.ndarray):
    z = np.concatenate(
        [np.asarray(z_i, np.float32), np.asarray(z_j, np.float32)], axis=0
    )
    in_maps = []
    for c in range(NCORES):
        zr = np.roll(z, -c * RPC, axis=0)
        in_maps.append(
            {
                "z_nat": np.ascontiguousarray(zr),
                "zT": np.ascontiguousarray(zr.T),
            }
        )
    return in_maps


def gather(results) -> np.ndarray:
    total = 0.0
    for res in results:
        total += res["loss"].astype(np.float64).sum()
    return np.float32(total / N)


def kernel(z_i: np.ndarray, z_j: np.ndarray, **run_kwargs) -> np.ndarray:
    from concourse.bass_utils import run_bass_kernel_spmd

    nc = get_nc()
    in_maps = make_in_maps(z_i, z_j)
    res = run_bass_kernel_spmd(
        nc, in_maps, core_ids=list(range(NCORES)), **run_kwargs
    )
    out = gather(res.results)
    kernel.last_results = res
    return out


# revision 21
# speedup vs baseline: 1.0793x; 1.0793x over previous
"""NT-Xent loss kernel for Trainium2 (8 NeuronCores, SPMD).

Math (derived from the reference):
  z = concat(z_i, z_j)                         (N=8192, D=128)
  zn = z / max(||z||, 1e-8)
  G[a,b] = zn[a].zn[b] / temp   (temp=0.5, so G = 2*cos-sim)
  per-row loss_a = log(sum_{i != p(a)} e^{G[a,i]}) - G[a,a]
                 = log(sum_all_i e^{G[a,i]} - e^{G[a,p(a)]}) - G[a,a]
  with p(a) = (a + 4096) % 8192;  loss = mean over rows.

Sharding: data-parallel row blocks.  Core c handles rows
[1024c, 1024c+1024).  Each core receives the FULL z, but np.roll'd by
-1024c rows so the SPMD program only uses static offsets: its row block
is always columns [0, 1024) of the (rotated) transposed operand, the
"diag" entry of block-row j is column j, and the masked positive column
is j+4096.  No collectives; the host sums 8 partial outputs.

Per-core device pipeline:
  1. DMA z [8192,128] as 64 natural tiles [128(row),128(d)] (SWDGE casts
     fp32 -> fp32r in flight).
  2. n2[p,t] = ||row 128t+p||^2 via DVE fused square-reduce / ACT
     Square+accum (split across both engines).
  3. r = n2^-0.5 computed as exp(-0.5*ln(n2)) — Ln/Exp/Square all live in
     one ACT table set, so no mid-kernel table switches for this.
  4. znT block t = z_tile_t.T @ diag(r_nat[:,t]) on the PE — transpose
     and column normalization in a single matmul; the diag operand is
     identity * r (one DVE tensor_scalar_mul).  PSUM results are copied
     to SBUF (alternating DVE/ACT) as the fp32r matmul operand znT.
  5. For each 128-row tile: PE matmul (fp32r) -> PSUM [128, 2048] blocks;
     ACT Exp(scale=2) with accum_out gives row sums of e^G; DVE
     mask-reduce extracts 2*diag and 2*pos from PSUM.
  6. loss_tile = ln(S - e^pos2) - diag2, DMA'd out as [128, 8] per core.
"""

import os

import numpy as np

N = 8192
D = 128
NCORES = 8
RPC = N // NCORES  # rows per core = 1024
NT = N // 128  # 64 natural z tiles
RT = RPC // 128  # 8 row tiles per core
CHUNK = 512
CB = 2048  # psum col-block
NCB = N // CB  # 4
NQ = 4  # input DMA chunks
TPQ = NT // NQ  # tiles per chunk = 16
HALF = N // 2

_NC_CACHE = {}


def _build_nc(mm_dtype: str, reps: int = 1):
    from contextlib import ExitStack

    import concourse.bass as bass  # noqa: F401
    import concourse.tile as tile
    from concourse import bacc, mybir
    from concourse.masks import make_identity

    f32 = mybir.dt.float32
    bf16 = mybir.dt.bfloat16
    FN = mybir.ActivationFunctionType
    ALU = mybir.AluOpType

    if mm_dtype == "f32r":
        mmdt = mybir.dt.float32r
    elif mm_dtype == "f32":
        mmdt = f32
    elif mm_dtype == "bf16":
        mmdt = bf16
    else:
        raise ValueError(mm_dtype)

    nc = bacc.Bacc(
        "TRN2", target_bir_lowering=False, debug=False, num_devices=NCORES
    )
    z_nat = nc.dram_tensor("z_nat", [N, D], f32, kind="ExternalInput")
    loss_out = nc.dram_tensor("loss", [128, RT], f32, kind="ExternalOutput")

    def body(pools, ident, const):
        big, small, scr, diagp, psum = pools

        # ---- input DMA: 4 chunks, SWDGE casting f32 -> mmdt in flight ----
        znat_sb = []
        z_nat_t = z_nat.ap().rearrange("(t p) d -> p t d", p=128)
        for q in range(NQ):
            t = big.tile([128, TPQ, 128], mmdt, tag=f"znat{q}")
            nc.gpsimd.dma_start(t[:], z_nat_t[:, q * TPQ : (q + 1) * TPQ, :])
            znat_sb.append(t)

        # ---- n2 per row, natural layout [128, 64] ----
        n2 = small.tile([128, NT], f32, tag="n2")
        ttr_scr = scr.tile([128, 128], f32, tag="ttr_scr")
        sq_scr = scr.tile([128, 128], f32, tag="sq_scr")
        for t in range(NT):
            q, i = divmod(t, TPQ)
            src = znat_sb[q][:, i, :]
            if t % 2 == 0:
                nc.vector.scalar_tensor_tensor(
                    out=ttr_scr[:],
                    in0=src,
                    scalar=1.0,
                    in1=src,
                    op0=ALU.mult,
                    op1=ALU.mult,
                    accum_out=n2[:, t : t + 1],
                )
            else:
                nc.scalar.activation(
                    out=sq_scr[:],
                    in_=src,
                    func=FN.Square,
                    accum_out=n2[:, t : t + 1],
                )

        # ---- r = exp(-0.5 * ln(max(n2, 1e-16))) = 1/sqrt(n2) ----
        nmax = small.tile([128, NT], f32, tag="nmax")
        nc.vector.tensor_scalar_max(out=nmax[:], in0=n2[:], scalar1=1e-16)
        lnn = small.tile([128, NT], f32, tag="lnn")
        nc.scalar.activation(out=lnn[:], in_=nmax[:], func=FN.Ln)
        r_nat = small.tile([128, NT], f32, tag="r_nat")
        nc.scalar.activation(out=r_nat[:], in_=lnn[:], func=FN.Exp, scale=-0.5)

        # ---- znT blocks: z_tile_t.T @ diag(r_t), 16 blocks per psum group,
        # copied to SBUF split across DVE/ACT ----
        znt = []  # 4 tensors [128, 2048]
        for g in range(NQ):
            zn_ps = psum.tile([128, CB], f32, tag="ps")
            for s in range(TPQ):
                t = g * TPQ + s
                dg = diagp.tile([128, 128], mmdt, tag=f"dg{t % 4}")
                nc.vector.tensor_scalar_mul(
                    out=dg[:], in0=ident[:], scalar1=r_nat[:, t : t + 1]
                )
                nc.tensor.matmul(
                    zn_ps[:, s * 128 : (s + 1) * 128],
                    znat_sb[g][:, s, :],
                    dg[:],
                    start=True,
                    stop=True,
                )
            zn_c = big.tile([128, CB], mmdt, tag=f"znt{g}")
            nc.vector.tensor_copy(out=zn_c[:, : CB // 2], in_=zn_ps[:, : CB // 2])
            nc.scalar.copy(out=zn_c[:, CB // 2 :], in_=zn_ps[:, CB // 2 :])
            znt.append(zn_c)

        # ---- main: G row blocks, exp row-sums, diag/pos extraction ----
        accs = small.tile([128, RT, NCB], f32, tag="accs")
        diag2 = small.tile([128, RT], f32, tag="diag2")
        pos2 = small.tile([128, RT], f32, tag="pos2")
        e_scr = scr.tile([128, CB], bf16, tag="e_scr")
        m_scr = scr.tile([128, 128], f32, tag="m_scr")

        for rt in range(RT):
            lhsT = znt[0][:, rt * 128 : (rt + 1) * 128]
            for cb in range(NCB):
                G_ps = psum.tile([128, CB], f32, tag="ps")
                for k in range(CB // CHUNK):
                    nc.tensor.matmul(
                        G_ps[:, k * CHUNK : (k + 1) * CHUNK],
                        lhsT,
                        znt[cb][:, k * CHUNK : (k + 1) * CHUNK],
                        start=True,
                        stop=True,
                    )
                if cb == 0:
                    nc.vector.scalar_tensor_tensor(
                        out=m_scr[:],
                        in0=G_ps[:, rt * 128 : rt * 128 + 128],
                        scalar=2.0,
                        in1=const,
                        op0=ALU.mult,
                        op1=ALU.mult,
                        accum_out=diag2[:, rt : rt + 1],
                    )
                if cb == 2:
                    off = HALF + rt * 128 - 2 * CB
                    nc.vector.scalar_tensor_tensor(
                        out=m_scr[:],
                        in0=G_ps[:, off : off + 128],
                        scalar=2.0,
                        in1=const,
                        op0=ALU.mult,
                        op1=ALU.mult,
                        accum_out=pos2[:, rt : rt + 1],
                    )
                nc.scalar.activation(
                    out=e_scr[:],
                    in_=G_ps[:],
                    func=FN.Exp,
                    scale=2.0,
                    accum_out=accs[:, rt, cb : cb + 1],
                )

        # ---- assembly: loss = ln(S - e^pos2) - diag2 ----
        S = small.tile([128, RT], f32, tag="S")
        nc.vector.reduce_sum(out=S[:], in_=accs[:], axis=mybir.AxisListType.X)
        P = small.tile([128, RT], f32, tag="P")
        nc.scalar.activation(out=P[:], in_=pos2[:], func=FN.Exp)
        T = small.tile([128, RT], f32, tag="T")
        nc.vector.tensor_sub(T[:], S[:], P[:])
        L = small.tile([128, RT], f32, tag="L")
        nc.scalar.activation(out=L[:], in_=T[:], func=FN.Ln)
        loss_sb = small.tile([128, RT], f32, tag="loss_sb")
        nc.vector.tensor_sub(loss_sb[:], L[:], diag2[:])
        nc.gpsimd.dma_start(loss_out.ap()[:, :], loss_sb[:])

    with tile.TileContext(nc) as tc, ExitStack() as ctx:
        const_pool = ctx.enter_context(tc.tile_pool(name="const", bufs=1))
        big = ctx.enter_context(tc.tile_pool(name="big", bufs=1))
        small = ctx.enter_context(tc.tile_pool(name="small", bufs=1))
        scr = ctx.enter_context(tc.tile_pool(name="scr", bufs=1))
        diagp = ctx.enter_context(tc.tile_pool(name="diag", bufs=1))
        psum = ctx.enter_context(tc.tile_pool(name="psum", bufs=2, space="PSUM"))

        ident = const_pool.tile([128, 128], f32, tag="ident")
        make_identity(nc, ident[:])

        for _rep in range(reps):
            body((big, small, scr, diagp, psum), ident, ident[:])

    nc.compile()
    return nc


def get_nc(reps: int = 1):
    mm_dtype = os.environ.get("NTX_MM_DTYPE", "f32r")
    key = (mm_dtype, reps)
    if key not in _NC_CACHE:
        _NC_CACHE[key] = _build_nc(mm_dtype, reps)
    return _NC_CACHE[key]


def make_in_maps(z_i: np.ndarray, z_j: np.ndarray):
    z = np.concatenate(
        [np.asarray(z_i, np.float32), np.asarray(z_j, np.float32)], axis=0
    )
    in_maps = []
    for c in range(NCORES):
        in_maps.append({"z_nat": np.ascontiguousarray(np.roll(z, -c * RPC, axis=0))})
    return in_maps


def gather(results) -> np.ndarray:
    total = 0.0
    for res in results:
        total += res["loss"].astype(np.float64).sum()
    return np.float32(total / N)


def kernel(z_i: np.ndarray, z_j: np.ndarray, **run_kwargs) -> np.ndarray:
    from concourse.bass_utils import run_bass_kernel_spmd

    nc = get_nc()
    in_maps = make_in_maps(z_i, z_j)
    res = run_bass_kernel_spmd(
        nc, in_maps, core_ids=list(range(NCORES)), **run_kwargs
    )
    out = gather(res.results)
    kernel.last_results = res
    return out


# revision 27
# speedup vs baseline: 1.2656x; 1.1726x over previous
"""NT-Xent loss kernel for Trainium2 (8 NeuronCores, SPMD).

Math (derived from the reference):
  z = concat(z_i, z_j)                         (N=8192, D=128)
  zn = z / max(||z||, 1e-8)
  G[a,b] = zn[a].zn[b] / temp   (temp=0.5, so G = 2*cos-sim)
  per-row loss_a = log(sum_{i != p(a)} e^{G[a,i]}) - G[a,a]
                 = log(sum_all_i e^{G[a,i]} - e^{G[a,p(a)]}) - G[a,a]
  with p(a) = (a + 4096) % 8192;  loss = mean over rows.

Sharding: data-parallel row blocks.  Core c handles rows
[1024c, 1024c+1024).  Each core receives the FULL z, but np.roll'd by
-1024c rows so the SPMD program only uses static offsets: its row block
is always columns [0, 1024) of the (rotated) transposed operand, the
"diag" entry of block-row j is column j, and the masked positive column
is j+4096.  No collectives; the host sums 8 partial outputs.

Per-core device pipeline:
  1. DMA z [8192,128] as 64 natural tiles [128(row),128(d)] (SWDGE casts
     fp32 -> fp32r in flight).
  2. n2[p,t] = ||row 128t+p||^2 via DVE fused square-reduce / ACT
     Square+accum (split across both engines).
  3. r = n2^-0.5 computed as exp(-0.5*ln(n2)) — Ln/Exp/Square all live in
     one ACT table set, so no mid-kernel table switches for this.
  4. znT block t = z_tile_t.T @ diag(r_nat[:,t]) on the PE — transpose
     and column normalization in a single matmul; the diag operand is
     identity * r (one DVE tensor_scalar_mul).  PSUM results are copied
     to SBUF (alternating DVE/ACT) as the fp32r matmul operand znT.
  5. For each 128-row tile: PE matmul (fp32r) -> PSUM [128, 2048] blocks;
     ACT Exp(scale=2) with accum_out gives row sums of e^G; DVE
     mask-reduce extracts 2*diag and 2*pos from PSUM.
  6. loss_tile = ln(S - e^pos2) - diag2, DMA'd out as [128, 8] per core.
"""

import os

import numpy as np

N = 8192
D = 128
NCORES = 8
RPC = N // NCORES  # rows per core = 1024
NT = N // 128  # 64 natural z tiles
RT = RPC // 128  # 8 row tiles per core
CHUNK = 512
CB = 2048  # psum col-block
NCB = N // CB  # 4
NQ = 4  # input DMA chunks
TPQ = NT // NQ  # tiles per chunk = 16
HALF = N // 2

_NC_CACHE = {}


def _build_nc(mm_dtype: str, reps: int = 1):
    from contextlib import ExitStack

    import concourse.bass as bass  # noqa: F401
    import concourse.tile as tile
    from concourse import bacc, mybir
    from concourse.masks import make_identity

    f32 = mybir.dt.float32
    bf16 = mybir.dt.bfloat16
    FN = mybir.ActivationFunctionType
    ALU = mybir.AluOpType

    if mm_dtype == "f32r":
        mmdt = mybir.dt.float32r
    elif mm_dtype == "f32":
        mmdt = f32
    elif mm_dtype == "bf16":
        mmdt = bf16
    else:
        raise ValueError(mm_dtype)

    nc = bacc.Bacc(
        "TRN2", target_bir_lowering=False, debug=False, num_devices=NCORES
    )
    z_nat = nc.dram_tensor("z_nat", [N, D], f32, kind="ExternalInput")
    loss_out = nc.dram_tensor("loss", [128, RT], f32, kind="ExternalOutput")

    def body(pools, ident, const):
        big, small, scr, diagp, psum = pools

        # ---- input DMA: 4 chunks, SWDGE casting f32 -> mmdt in flight ----
        znat_sb = []
        z_nat_t = z_nat.ap().rearrange("(t p) d -> p t d", p=128)
        for q in range(NQ):
            t = big.tile([128, TPQ, 128], mmdt, tag=f"znat{q}")
            nc.gpsimd.dma_start(t[:], z_nat_t[:, q * TPQ : (q + 1) * TPQ, :])
            znat_sb.append(t)

        # ---- prefix, pipelined per input chunk q: n2 (squares split
        # DVE/ACT) -> r chunk -> znT blocks via z_tile.T @ diag(r) -> SBUF.
        # rt=0's G block for chunk q is interleaved right after chunk q so
        # ScalarE starts exp'ing as early as possible. ----
        n2 = small.tile([128, NT], f32, tag="n2")
        ttr_scr = scr.tile([128, 128], f32, tag="ttr_scr")
        sq_scr = scr.tile([128, 128], f32, tag="sq_scr")
        nmax = small.tile([128, NT], f32, tag="nmax")
        r_nat = small.tile([128, NT], f32, tag="r_nat")
        accs = small.tile([128, RT, NCB], f32, tag="accs")
        diag2 = small.tile([128, RT], f32, tag="diag2")
        pos2 = small.tile([128, RT], f32, tag="pos2")
        e_scr = scr.tile([128, CB], bf16, tag="e_scr")
        m_scr = scr.tile([128, 128], f32, tag="m_scr")
        znt = []  # 4 tensors [128, 2048]

        def g_block(rt, cb):
            lhsT = znt[0][:, rt * 128 : (rt + 1) * 128]
            G_ps = psum.tile([128, CB], f32, tag="ps")
            for k in range(CB // CHUNK):
                nc.tensor.matmul(
                    G_ps[:, k * CHUNK : (k + 1) * CHUNK],
                    lhsT,
                    znt[cb][:, k * CHUNK : (k + 1) * CHUNK],
                    start=True,
                    stop=True,
                )
            if cb == 0:
                nc.vector.scalar_tensor_tensor(
                    out=m_scr[:],
                    in0=G_ps[:, rt * 128 : rt * 128 + 128],
                    scalar=2.0,
                    in1=const,
                    op0=ALU.mult,
                    op1=ALU.mult,
                    accum_out=diag2[:, rt : rt + 1],
                )
            if cb == 2:
                off = HALF + rt * 128 - 2 * CB
                nc.vector.scalar_tensor_tensor(
                    out=m_scr[:],
                    in0=G_ps[:, off : off + 128],
                    scalar=2.0,
                    in1=const,
                    op0=ALU.mult,
                    op1=ALU.mult,
                    accum_out=pos2[:, rt : rt + 1],
                )
            nc.scalar.activation(
                out=e_scr[:],
                in_=G_ps[:],
                func=FN.Exp,
                scale=2.0,
                accum_out=accs[:, rt, cb : cb + 1],
            )

        for q in range(NQ):
            for i in range(TPQ):
                t = q * TPQ + i
                src = znat_sb[q][:, i, :]
                if i % 8 < 5:
                    nc.vector.scalar_tensor_tensor(
                        out=ttr_scr[:],
                        in0=src,
                        scalar=1.0,
                        in1=src,
                        op0=ALU.mult,
                        op1=ALU.mult,
                        accum_out=n2[:, t : t + 1],
                    )
                else:
                    nc.scalar.activation(
                        out=sq_scr[:],
                        in_=src,
                        func=FN.Square,
                        accum_out=n2[:, t : t + 1],
                    )
            sl = slice(q * TPQ, (q + 1) * TPQ)
            nc.vector.tensor_scalar_max(
                out=nmax[:, sl], in0=n2[:, sl], scalar1=1e-16
            )
            lnn = scr.tile([128, TPQ], f32, tag=f"lnn{q % 2}")
            nc.scalar.activation(out=lnn[:], in_=nmax[:, sl], func=FN.Ln)
            nc.scalar.activation(
                out=r_nat[:, sl], in_=lnn[:], func=FN.Exp, scale=-0.5
            )

            zn_ps = psum.tile([128, CB], f32, tag="ps")
            for s in range(TPQ):
                t = q * TPQ + s
                dg = diagp.tile([128, 128], mmdt, tag=f"dg{t % 4}")
                nc.vector.tensor_scalar_mul(
                    out=dg[:], in0=ident[:], scalar1=r_nat[:, t : t + 1]
                )
                nc.tensor.matmul(
                    zn_ps[:, s * 128 : (s + 1) * 128],
                    znat_sb[q][:, s, :],
                    dg[:],
                    start=True,
                    stop=True,
                )
            zn_c = big.tile([128, CB], mmdt, tag=f"znt{q}")
            nc.vector.tensor_copy(out=zn_c[:], in_=zn_ps[:])
            znt.append(zn_c)
            if q > 0:
                # rt=0 col-block q-? : znt[0] exists once q>=1; emit the
                # G block for each finished chunk to keep ACT busy early
                g_block(0, q - 1)
        g_block(0, NQ - 1)

        # ---- main: remaining G row blocks ----
        for rt in range(1, RT):
            for cb in range(NCB):
                g_block(rt, cb)

        # ---- assembly: loss = ln(S - e^pos2) - diag2 ----
        S = small.tile([128, RT], f32, tag="S")
        nc.vector.reduce_sum(out=S[:], in_=accs[:], axis=mybir.AxisListType.X)
        P = small.tile([128, RT], f32, tag="P")
        nc.scalar.activation(out=P[:], in_=pos2[:], func=FN.Exp)
        T = small.tile([128, RT], f32, tag="T")
        nc.vector.tensor_sub(T[:], S[:], P[:])
        L = small.tile([128, RT], f32, tag="L")
        nc.scalar.activation(out=L[:], in_=T[:], func=FN.Ln)
        loss_sb = small.tile([128, RT], f32, tag="loss_sb")
        nc.vector.tensor_sub(loss_sb[:], L[:], diag2[:])
        nc.gpsimd.dma_start(loss_out.ap()[:, :], loss_sb[:])

    # Pin every ACT function to the one table set that contains all of
    # Ln/Exp/Square/Copy (natural_log_exp_and_others): present the
    # table-load pass a view where only that set is non-empty, so it never
    # inserts mid-kernel table switches (each costs ~2.7us on ScalarE).
    import concourse.hw_specs as hw_specs

    _real_tables = hw_specs.get_activation_tables(nc.m.arch)
    _pruned = {
        name: (fns if name == "natural_log_exp_and_others" else set())
        for name, fns in _real_tables.items()
    }
    _orig_get_tables = bacc.get_activation_tables

    with tile.TileContext(nc) as tc, ExitStack() as ctx:
        const_pool = ctx.enter_context(tc.tile_pool(name="const", bufs=1))
        big = ctx.enter_context(tc.tile_pool(name="big", bufs=1))
        small = ctx.enter_context(tc.tile_pool(name="small", bufs=1))
        scr = ctx.enter_context(tc.tile_pool(name="scr", bufs=1))
        diagp = ctx.enter_context(tc.tile_pool(name="diag", bufs=1))
        psum = ctx.enter_context(tc.tile_pool(name="psum", bufs=2, space="PSUM"))

        ident = const_pool.tile([128, 128], f32, tag="ident")
        make_identity(nc, ident[:])

        for _rep in range(reps):
            body((big, small, scr, diagp, psum), ident, ident[:])

    bacc.get_activation_tables = lambda arch: _pruned
    try:
        nc.compile()
    finally:
        bacc.get_activation_tables = _orig_get_tables
    return nc


def get_nc(reps: int = 1):
    mm_dtype = os.environ.get("NTX_MM_DTYPE", "f32r")
    key = (mm_dtype, reps)
    if key not in _NC_CACHE:
        _NC_CACHE[key] = _build_nc(mm_dtype, reps)
    return _NC_CACHE[key]


def make_in_maps(z_i: np.ndarray, z_j: np.ndarray):
    z = np.concatenate(
        [np.asarray(z_i, np.float32), np.asarray(z_j, np.float32)], axis=0
    )
    in_maps = []
    for c in range(NCORES):
        in_maps.append({"z_nat": np.ascontiguousarray(np.roll(z, -c * RPC, axis=0))})
    return in_maps


def gather(results) -> np.ndarray:
    total = 0.0
    for res in results:
        total += res["loss"].astype(np.float64).sum()
    return np.float32(total / N)


def kernel(z_i: np.ndarray, z_j: np.ndarray, **run_kwargs) -> np.ndarray:
    from concourse.bass_utils import run_bass_kernel_spmd

    nc = get_nc()
    in_maps = make_in_maps(z_i, z_j)
    res = run_bass_kernel_spmd(
        nc, in_maps, core_ids=list(range(NCORES)), **run_kwargs
    )
    out = gather(res.results)
    kernel.last_results = res
    return out
